# revision 47
# baseline (speedup 1.0000x reference)
"""Multi-head self-attention (B=2, T=2048, C=1024, H=16) on 8 trn2 cores.

Sharding: core c -> batch b = c//4, heads 4*(c%4) .. 4*(c%4)+3.
Each core: QKV projection for its 4 heads, causal attention in S^T layout
(keys on partitions), partial output projection over its heads' rows of Wo.
Host sums the 4 partials per batch element and adds bo.

All matmul operands are bf16 (PSUM accumulation stays fp32). The causal
mask is folded into the S logits pre-exp via an identity-stationary matmul
that accumulates -30000 into masked PSUM positions, so exp produces exact
zeros and no post-exp masking pass is needed. Softmax denominators come
from an appended ones-column in the V operand; the division is a
reciprocal + partition-broadcast + fused PSUM multiply on vector/gpsimd.

The emission is a software pipeline: each projection quarter is followed
by the attention slab it unblocks (causal => slab s needs only quarters
<= s), so the scalar-engine exp stream starts ~20us in and overlaps all
remaining projection/output matmuls. Output-projection blocks are woven
into later slabs as PE filler, input DMA descriptor issue is spread over
the three DMA-capable engine queues, and S/PV run skewed by two chunk
groups so exp latency stays off the PE critical path. PSUM: one shared
2-buf pool for projection+outproj psums, 2x2 banks for S tiles, 2 for
PV accumulators.
"""
import sys

sys.path.insert(0, "/opt/trn_rl_repo")

import numpy as np
import ml_dtypes

BF16NP = ml_dtypes.bfloat16

B, T, C, H = 2, 2048, 1024, 16
HD = C // H            # 64
NCORES = 8
HPC = H // (NCORES // B)   # heads per core = 4
QB = 128               # q block (columns of S^T)
KB = 128               # k chunk (partitions of S^T)
NJ = T // KB           # 16
NI = T // QB           # 16
SLAB = 512             # q columns processed per attention pass
NSLAB = T // SLAB      # 4
BPS = SLAB // QB       # q blocks per slab = 4
CI = C // 128          # 8 contraction chunks for projections
SCALE = HD ** -0.5
MASKNEG = -30000.0

_cache = {}
_DEBUG = False


def _build_plan(mask_bool: np.ndarray):
    """mask_bool: [T, T] (q, k). Returns per (j, i) block types and tiles.

    type 0 = all valid (no mask work), 1 = all masked (skip), 2 = mixed.
    Tiles are stored transposed to match S^T ([k_local, q_local])."""
    btype = np.zeros((NJ, NI), dtype=np.int32)
    tidx = np.full((NJ, NI), -1, dtype=np.int32)
    tiles = []
    tile_map = {}

    def add_tile(sub):
        key = sub.tobytes()
        if key not in tile_map:
            tile_map[key] = len(tiles)
            tiles.append(sub.T.astype(np.float32))
        return tile_map[key]

    for j in range(NJ):
        for i in range(NI):
            sub = mask_bool[i * QB:(i + 1) * QB, j * KB:(j + 1) * KB]
            if sub.all():
                btype[j, i] = 0
            elif not sub.any():
                btype[j, i] = 1
            else:
                btype[j, i] = 2
                tidx[j, i] = add_tile(sub)
    # all-masked blocks inside a live run still need a full -inf tile
    for j in range(NJ):
        for s in range(NSLAB):
            i_lo, i_hi = s * BPS, (s + 1) * BPS
            live = [i for i in range(i_lo, i_hi) if btype[j, i] != 1]
            if live:
                for i in range(min(live), max(live) + 1):
                    if btype[j, i] == 1:
                        btype[j, i] = 2
                        tidx[j, i] = add_tile(
                            np.zeros((QB, KB), dtype=bool))
    if not tiles:
        tiles.append(np.ones((KB, QB), dtype=np.float32))
    return btype, tidx, np.stack(tiles)


def _build_program(btype, tidx, n_tiles, apply_qk_bias, apply_v_bias):
    import concourse.bass as bass
    import concourse.tile as tile
    import concourse.mybir as mybir
    from concourse import bacc

    F32 = mybir.dt.float32
    BF16 = mybir.dt.bfloat16
    AF = mybir.ActivationFunctionType
    MULT = mybir.AluOpType.mult

    nc = bacc.Bacc("TRN2", target_bir_lowering=False, debug=False)
    xt_d = nc.dram_tensor("xt", [C, T], BF16, kind="ExternalInput").ap()
    wqk_d = nc.dram_tensor("wqk", [C, 4 * 128], BF16, kind="ExternalInput").ap()
    wv_d = nc.dram_tensor("wv", [C, HPC * HD], BF16, kind="ExternalInput").ap()
    wo_d = nc.dram_tensor("wo", [HPC * HD, C], BF16, kind="ExternalInput").ap()
    mask_d = nc.dram_tensor("masks", [n_tiles, KB, QB], BF16,
                            kind="ExternalInput").ap()
    ident_d = nc.dram_tensor("ident", [128, 128], BF16,
                             kind="ExternalInput").ap()
    if apply_qk_bias:
        bqk_d = nc.dram_tensor("bqk", [128, 4], F32, kind="ExternalInput").ap()
    if apply_v_bias:
        bv_d = nc.dram_tensor("bv", [128, 2], F32, kind="ExternalInput").ap()
    out_d = nc.dram_tensor("out", [T, C], BF16, kind="ExternalOutput").ap()
    if _DEBUG:
        dbg = {
            "dbg_qp0": nc.dram_tensor("dbg_qp0", [128, T], BF16,
                                      kind="ExternalOutput").ap(),
            "dbg_kz0": nc.dram_tensor("dbg_kz0", [128, T], BF16,
                                      kind="ExternalOutput").ap(),
            "dbg_kz1": nc.dram_tensor("dbg_kz1", [128, T], BF16,
                                      kind="ExternalOutput").ap(),
            "dbg_vaug": nc.dram_tensor("dbg_vaug", [128, NJ * HPC * 128],
                                       BF16, kind="ExternalOutput").ap(),
            "dbg_attn0": nc.dram_tensor("dbg_attn0", [128, T], BF16,
                                        kind="ExternalOutput").ap(),
            "dbg_sums": nc.dram_tensor("dbg_sums", [1, 4 * T], F32,
                                       kind="ExternalOutput").ap(),
            "dbg_praw": nc.dram_tensor("dbg_praw", [128, T], F32,
                                       kind="ExternalOutput").ap(),
        }

    with tile.TileContext(nc) as tc:
        with tc.tile_pool(name="weights", bufs=1) as wpool, \
             tc.tile_pool(name="acts", bufs=1) as apool:
            # ---- resident SBUF tensors ----
            xt = wpool.tile([128, CI, T], BF16)        # x^T, c_in chunked
            wqk = wpool.tile([128, CI, 512], BF16)
            wv = wpool.tile([128, CI, HPC * HD], BF16)
            wo = wpool.tile([128, 2, C], BF16)         # head-pair chunks
            masks = wpool.tile([128, n_tiles * QB], BF16)   # -30000/0 tiles
            ident = wpool.tile([128, 128], BF16)
            bias_m2 = wpool.tile([128, 1], F32)        # exp bias: -3.0
            if apply_qk_bias:
                bqk = wpool.tile([128, 4], F32)
            if apply_v_bias:
                bv = wpool.tile([128, 2], F32)
            # q tiles hold (q_hA | q_hB) on partitions 0-63 / 64-127.
            # k is stored zero-padded per head (other head's partitions are
            # zero) so S matmuls present K=128 to the PE — K=64 matmuls do
            # not register as HAM activity and leave the clock at 1.2 GHz.
            qp = [apool.tile([128, T], BF16, tag=f"qp{i}", name=f"qp{i}")
                  for i in range(2)]
            kz = [apool.tile([128, T], BF16, tag=f"kz{i}", name=f"kz{i}")
                  for i in range(4)]          # index = 2*pair + head
            # per (k-chunk, head) a 128-col slot: v(64) | ones | zero pad
            vaug = apool.tile([128, NJ, HPC, 128], BF16)
            attn = [apool.tile([128, T], BF16, tag=f"attn{p}",
                               name=f"attn{p}") for p in range(2)]
            if _DEBUG:
                dbg_sums_sb = apool.tile([1, 4 * T], F32, tag="dbgsums",
                                         name="dbgsums")
                dbg_praw_sb = apool.tile([128, T], F32, tag="dbgpraw",
                                         name="dbgpraw")

            # ---- input DMAs: issue spread across engine queues so the
            # descriptor-issue serialization (~0.6us each) does not gate the
            # first projection matmuls. x quarters are issued quarter-major.
            for ci in range(CI):
                nc.sync.dma_start(wqk[:, ci, :],
                                  wqk_d[ci * 128:(ci + 1) * 128, :])
            for ci in range(CI):
                nc.sync.dma_start(wv[:, ci, :],
                                  wv_d[ci * 128:(ci + 1) * 128, :])
            x_eng = [None, nc.gpsimd, nc.sync, nc.gpsimd]
            for qn in range(4):
                sl = slice(qn * SLAB, (qn + 1) * SLAB)
                for ci in range(CI):
                    eng = x_eng[qn]
                    if qn == 0:
                        eng = nc.scalar if ci % 2 == 0 else nc.gpsimd
                    eng.dma_start(xt[:, ci, sl],
                                  xt_d[ci * 128:(ci + 1) * 128, sl])
            for t in range(n_tiles):
                nc.sync.dma_start(masks[:, t * QB:(t + 1) * QB], mask_d[t])
            nc.sync.dma_start(ident[:], ident_d)
            if apply_qk_bias:
                nc.sync.dma_start(bqk[:], bqk_d)
            if apply_v_bias:
                nc.sync.dma_start(bv[:], bv_d)
            nc.sync.dma_start(wo[:, 0, :], wo_d[0:128, :])
            nc.sync.dma_start(wo[:, 1, :], wo_d[128:256, :])
            # zero halves of kz + vaug padding + ones column of vaug
            va = vaug
            for p in range(2):
                nc.vector.memset(kz[2 * p][64:128, :], 0.0)
                nc.vector.memset(kz[2 * p + 1][0:64, :], 0.0)
            nc.vector.memset(va[:, :, :, HD + 1:], 0.0)
            nc.vector.memset(bias_m2[:], -3.0)
            nc.vector.tensor_copy(
                va[:, :, :, HD:HD + 1],
                nc.const_aps.tensor(1.0, (128, NJ, HPC, 1)))

            # ---- staggered pipeline ----
            # Emission order interleaves projection quarters with attention
            # slabs (slab s only needs q/k/v up to quarter s, causal) and
            # output projection, so the scalar-engine exp stream starts ~25us
            # in and overlaps the remaining projection matmuls. One shared
            # PSUM tag serves projection and output-projection psums (they
            # never overlap in time), keeping the total at 8 banks:
            # big 2x1 + sps 2x2 + outps 2x1.
            with tc.tile_pool(name="ppool", bufs=2, space="PSUM") as ppool, \
                 tc.tile_pool(name="psattn", bufs=2, space="PSUM") as sp, \
                 tc.tile_pool(name="psout", bufs=1, space="PSUM") as op, \
                 tc.tile_pool(name="ptp", bufs=6) as ptp, \
                 tc.tile_pool(name="divp", bufs=3) as divp, \
                 tc.tile_pool(name="osb", bufs=4) as osb:

                def emit_proj_quarter(qn):
                    kv_eng = nc.scalar if qn < 3 else nc.vector
                    sl = slice(qn * SLAB, (qn + 1) * SLAB)
                    for co in (1, 3, "v", 0, 2):
                        if co == "v":
                            for tj in range(qn * 4, qn * 4 + 4):
                                psv = ppool.tile([128, 512], F32, tag="big",
                                                 name="psv")
                                for ci in range(CI):
                                    nc.tensor.matmul(
                                        psv[:, 0:HPC * HD],
                                        xt[:, ci, tj * 128:(tj + 1) * 128],
                                        wv[:, ci, :],
                                        start=(ci == 0), stop=(ci == CI - 1))
                                pvr = psv[:, 0:HPC * HD].rearrange(
                                    "p (h d) -> p h d", h=HPC)
                                kv_eng_copy(kv_eng, va[:, tj, :, 0:HD], pvr)
                            continue
                        pair, is_k = co // 2, co % 2
                        ps = ppool.tile([128, 512], F32, tag="big", name="ps")
                        for ci in range(CI):
                            nc.tensor.matmul(
                                ps[:],
                                wqk[:, ci, co * 128:(co + 1) * 128],
                                xt[:, ci, sl],
                                start=(ci == 0), stop=(ci == CI - 1))
                        if is_k:
                            dsts = [(kz[2 * pair][0:64, sl], ps[0:64, :],
                                     (0, 64)),
                                    (kz[2 * pair + 1][64:128, sl],
                                     ps[64:128, :], (64, 128))]
                        else:
                            dsts = [(qp[pair][:, sl], ps[:], (0, 128))]
                        for dst_ap, src_ap, (b0, b1) in dsts:
                            if apply_qk_bias:
                                nc.scalar.activation(dst_ap, src_ap,
                                                     AF.Identity,
                                                     bias=bqk[b0:b1, co:co + 1],
                                                     scale=1.0)
                            elif is_k:
                                kv_eng_copy(kv_eng, dst_ap, src_ap)
                            else:
                                nc.vector.tensor_copy(dst_ap, src_ap)

                def kv_eng_copy(eng, dst_ap, src_ap):
                    if eng is nc.scalar:
                        nc.scalar.activation(dst_ap, src_ap, AF.Copy,
                                             bias=0.0)
                    else:
                        nc.vector.tensor_copy(dst_ap, src_ap)

                def outproj_ts(ts, last=False):
                    def emit():
                        for n0 in range(0, C, 512):
                            ps = ppool.tile([128, 512], F32, tag="big",
                                            name="opps")
                            for pair in range(2):
                                nc.tensor.matmul(
                                    ps[:],
                                    attn[pair][:, ts * 128:(ts + 1) * 128],
                                    wo[:, pair, n0:n0 + 512],
                                    start=(pair == 0), stop=(pair == 1))
                            ot = osb.tile([128, 512], BF16, tag="ot",
                                          name="ot")
                            if last:
                                # exp stream is finished; use the scalar
                                # engine so vector stays free for divisions
                                nc.scalar.activation(ot[:], ps[:], AF.Copy,
                                                     bias=0.0)
                            else:
                                nc.vector.tensor_copy(ot[:], ps[:])
                            nc.sync.dma_start(
                                out_d[ts * 128:(ts + 1) * 128, n0:n0 + 256],
                                ot[:, 0:256])
                            nc.sync.dma_start(
                                out_d[ts * 128:(ts + 1) * 128,
                                      n0 + 256:n0 + 512],
                                ot[:, 256:512])
                    return emit

                def emit_outproj(s, last=False):
                    for ts in range(s * BPS, (s + 1) * BPS):
                        outproj_ts(ts, last)()

                def emit_attn_pair(s, pair, fillers=()):
                    fillers = list(fillers)
                    i_lo, i_hi = s * BPS, (s + 1) * BPS
                    chunks = []
                    for j in range(NJ):
                        live = [i for i in range(i_lo, i_hi)
                                if btype[j, i] != 1]
                        if live:
                            chunks.append((j, min(live), max(live)))
                    if True:
                        q_t = qp[pair]
                        out_ps = [op.tile([HD + 2, SLAB], F32,
                                          tag=f"outps{hl}",
                                          name=f"outps{hl}", bufs=1)
                                  for hl in range(2)]
                        written = np.zeros(BPS, dtype=bool)
                        # group pairs of consecutive full-slab chunks so the
                        # PV accumulation runs as one fp8 DoubleRow matmul
                        # with a 256-deep contraction (half the PE row count)
                        groups = []
                        for (j, i0, i1) in chunks:
                            full = (i0 == i_lo and i1 == i_hi - 1)
                            groups.append([(j, i0, i1, full)])
                        pend = []  # [(group, pt tile, segs list, last)]

                        def emit_pv(prev):
                            group, pt, segs_l, last = prev
                            if len(group) == 2:
                                j0 = group[0][0]
                                (c0, c1, st_flag) = segs_l[0][0]
                                for hl in range(2):
                                    hh = 2 * pair + hl
                                    nc.tensor.matmul(
                                        out_ps[hl][:, c0:c1],
                                        vaug[:, j0:j0 + 2, hh, 0:HD + 2],
                                        pt[:, hl, :, c0:c1],
                                        start=st_flag, stop=last,
                                        perf_mode=DR,
                                        skip_group_check=True)
                                return
                            (j, i0, i1, full) = group[0]
                            r0 = i0 - i_lo
                            for hl in range(2):
                                hh = 2 * pair + hl
                                for (c0, c1, st_flag) in segs_l[0]:
                                    nc.tensor.matmul(
                                        out_ps[hl][0:HD + 1, c0:c1],
                                        vaug[:, j, hh, 0:HD + 1],
                                        pt[:, hl, 0,
                                           c0 - r0 * QB:c1 - r0 * QB],
                                        start=st_flag, stop=last,
                                        skip_group_check=True)

                        for gn, group in enumerate(groups):
                            pt = ptp.tile([128, 2, 2, SLAB], BF16, tag="pt",
                                          name="pt")
                            segs_l = []
                            for slot, (j, i0, i1, full) in enumerate(group):
                                n_cols = (i1 - i0 + 1) * QB
                                r0 = i0 - i_lo
                                mixed = [i for i in range(i0, i1 + 1)
                                         if btype[j, i] == 2]
                                sps = sp.tile([128, 2, SLAB], F32,
                                              tag="sst", name="sst")
                                for hl in range(2):
                                    nc.tensor.matmul(
                                        sps[:, hl, 0:n_cols],
                                        kz[2 * pair + hl][:,
                                                          j * KB:(j + 1) * KB],
                                        q_t[:, i0 * QB:i0 * QB + n_cols],
                                        start=True, stop=(not mixed))
                                for mi, i in enumerate(mixed):
                                    rel = (i - i0) * QB
                                    ti = tidx[j, i]
                                    for hl in range(2):
                                        nc.tensor.matmul(
                                            sps[:, hl, rel:rel + QB],
                                            ident[:],
                                            masks[:, ti * QB:(ti + 1) * QB],
                                            start=False,
                                            stop=(mi == len(mixed) - 1))
                                # bias -3 rescales all exp values by e^-3
                                # (cancels in the softmax division)
                                nc.scalar.activation(pt[:, :, slot, 0:n_cols],
                                                     sps[:, :, 0:n_cols],
                                                     AF.Exp, scale=SCALE,
                                                     bias=bias_m2[:])
                                # PV segments (split on first-write runs)
                                segs = []
                                c = r0 * QB
                                end = (i1 - i_lo + 1) * QB
                                while c < end:
                                    st = written[c // QB]
                                    cc = c + QB
                                    while cc < end and written[cc // QB] == st:
                                        cc += QB
                                    segs.append((c, cc, not st))
                                    c = cc
                                for rr in range(r0, i1 - i_lo + 1):
                                    written[rr] = True
                                segs_l.append(segs)
                            if len(pend) >= 2:
                                emit_pv(pend.pop(0))
                                if fillers:
                                    fillers.pop(0)()
                            pend.append((group, pt, segs_l,
                                         gn == len(groups) - 1))
                        for pg in pend:
                            emit_pv(pg)
                        for f in fillers:
                            f()

                        # softmax division: reciprocal of the ones-row sums,
                        # broadcast, then one fused PSUM x bcast -> bf16 mult
                        scols = slice(s * SLAB, (s + 1) * SLAB)
                        sums2 = divp.tile([1, 2, SLAB], F32, tag="sums2",
                                          name="sums2")
                        rec2 = divp.tile([1, 2, SLAB], F32, tag="rec2",
                                         name="rec2")
                        rec128 = divp.tile([128, 2, SLAB], F32, tag="rec128",
                                           name="rec128")
                        for hl in range(2):
                            nc.vector.tensor_copy(sums2[0:1, hl, :],
                                                  out_ps[hl][HD:HD + 1, :])
                        nc.vector.reciprocal_approx_fast(rec2[:], sums2[:])
                        for hl in range(2):
                            if _DEBUG:
                                row = 2 * pair + hl
                                nc.vector.tensor_copy(
                                    dbg_sums_sb[0:1,
                                                row * T + s * SLAB:
                                                row * T + (s + 1) * SLAB],
                                    out_ps[hl][HD:HD + 1, :])
                                if pair == 0:
                                    nc.vector.tensor_copy(
                                        dbg_praw_sb[64 * hl:64 * hl + 64,
                                                    scols],
                                        out_ps[hl][0:HD, :])
                            dst = attn[pair][64 * hl:64 * hl + 64, scols]
                            nc.gpsimd.partition_broadcast(rec128[:, hl, :],
                                                          rec2[0:1, hl, :])
                            rbc = rec128[64 * hl:64 * hl + 64, hl, :]
                            nc.vector.tensor_tensor(
                                out=dst, in0=out_ps[hl][0:HD, :], in1=rbc,
                                op=MULT)
                            if apply_v_bias:
                                nc.vector.tensor_scalar(
                                    out=dst, in0=dst,
                                    scalar1=bv[64 * hl:64 * hl + 64,
                                               pair:pair + 1],
                                    scalar2=None, op0=mybir.AluOpType.add)

                emit_proj_quarter(0)
                emit_proj_quarter(1)
                emit_attn_pair(0, 0)
                emit_attn_pair(0, 1)
                emit_proj_quarter(2)
                emit_attn_pair(1, 0)
                emit_attn_pair(1, 1)
                emit_proj_quarter(3)
                emit_attn_pair(2, 0)
                emit_attn_pair(2, 1)
                emit_attn_pair(3, 0, fillers=[
                    outproj_ts(ts) for ts in range(0, BPS)])
                emit_attn_pair(3, 1, fillers=[
                    outproj_ts(ts) for ts in range(BPS, 3 * BPS)])
                emit_outproj(3, last=True)
                if _DEBUG:
                    nc.sync.dma_start(dbg["dbg_qp0"], qp[0][:])
                    nc.sync.dma_start(dbg["dbg_kz0"], kz[0][:])
                    nc.sync.dma_start(dbg["dbg_kz1"], kz[1][:])
                    nc.sync.dma_start(
                        dbg["dbg_vaug"],
                        vaug[:].rearrange("p j h c -> p (j h c)"))
                    nc.sync.dma_start(dbg["dbg_attn0"], attn[0][:])
                    nc.sync.dma_start(dbg["dbg_sums"], dbg_sums_sb[:])
                    nc.sync.dma_start(dbg["dbg_praw"], dbg_praw_sb[:])

    nc.compile()
    return nc


def _get_program(mask_bool, apply_qk_bias, apply_v_bias):
    key = (mask_bool.tobytes(), apply_qk_bias, apply_v_bias)
    if key not in _cache:
        btype, tidx, tiles = _build_plan(mask_bool)
        nc = _build_program(btype, tidx, len(tiles), apply_qk_bias,
                            apply_v_bias)
        _cache[key] = (nc, tiles)
    return _cache[key]


def kernel(x, attention_mask, Wqkv, bqkv, Wo, bo, _trace=False):
    from concourse.bass_utils import run_bass_kernel_spmd

    x = np.asarray(x, dtype=np.float32)
    mask_bool = np.asarray(attention_mask)[0, 0] != 0
    Wqkv = np.asarray(Wqkv, dtype=np.float32)
    bqkv = np.asarray(bqkv, dtype=np.float32)
    Wo = np.asarray(Wo, dtype=np.float32)
    bo = np.asarray(bo, dtype=np.float32)

    apply_qk_bias = bool(np.any(bqkv[:2 * C]))
    apply_v_bias = bool(np.any(bqkv[2 * C:]))
    nc, tiles = _get_program(mask_bool, apply_qk_bias, apply_v_bias)

    maskneg = ((1.0 - tiles) * MASKNEG).astype(BF16NP)
    ident = np.eye(128, dtype=BF16NP)
    xts = [x[b].T.astype(BF16NP) for b in range(B)]
    in_maps = []
    for c in range(NCORES):
        b, g = divmod(c, NCORES // B)
        hs = [HPC * g + i for i in range(HPC)]
        # wqk column chunks: [q_h0|q_h1, k_h0|k_h1, q_h2|q_h3, k_h2|k_h3]
        cols, bias_cols = [], []
        for pair in range(2):
            ha, hb = hs[2 * pair], hs[2 * pair + 1]
            for base in (0, C):  # q then k offset in Wqkv columns
                cols.append(Wqkv[:, base + ha * HD:base + (ha + 1) * HD])
                cols.append(Wqkv[:, base + hb * HD:base + (hb + 1) * HD])
                bias_cols.append(np.concatenate([
                    bqkv[base + ha * HD:base + (ha + 1) * HD],
                    bqkv[base + hb * HD:base + (hb + 1) * HD]]))
        wqk_c = np.concatenate(cols, axis=1).astype(BF16NP)
        wv_c = np.concatenate(
            [Wqkv[:, 2 * C + h * HD:2 * C + (h + 1) * HD] for h in hs],
            axis=1).astype(BF16NP)
        wo_c = np.concatenate(
            [Wo[h * HD:(h + 1) * HD, :] for h in hs], axis=0).astype(BF16NP)
        im = {
            "xt": xts[b], "wqk": wqk_c, "wv": wv_c,
            "wo": wo_c, "masks": maskneg, "ident": ident,
        }
        if apply_qk_bias:
            im["bqk"] = np.stack(bias_cols, axis=1).astype(np.float32)
        if apply_v_bias:
            bv_c = np.zeros((128, 2), dtype=np.float32)
            for pair in range(2):
                ha, hb = hs[2 * pair], hs[2 * pair + 1]
                bv_c[0:HD, pair] = bqkv[2 * C + ha * HD:2 * C + (ha + 1) * HD]
                bv_c[HD:128, pair] = bqkv[2 * C + hb * HD:2 * C + (hb + 1) * HD]
            im["bv"] = bv_c
        in_maps.append(im)

    kwargs = {}
    if _trace:
        kwargs = dict(trace=True, trace_cores=[0])
    res = run_bass_kernel_spmd(nc, in_maps, core_ids=list(range(NCORES)),
                               **kwargs)
    out = np.empty((B, T, C), dtype=np.float32)
    gpb = NCORES // B
    for b in range(B):
        acc = res.results[b * gpb]["out"].astype(np.float32)
        for g in range(1, gpb):
            acc = acc + res.results[b * gpb + g]["out"].astype(np.float32)
        out[b] = acc + bo
    if _trace:
        kernel._last_results = res
    return out
